# revision 2
# baseline (speedup 1.0000x reference)
"""Fused multi-head attention for Trainium2 (Bass/Tile), 8-core SPMD. v5.

Problem: B=2, H=16, S=4096, D=64, fp32, mask == all-ones (unmasked softmax).

v2 changes vs baseline:
  * bf16 operands everywhere (K^T, Q^T, V', P^T): halves DMA/SBUF traffic and
    runs the PE at 1 cycle/row (fp32r measured ~2x slower per moving row).
  * exp split across two engines: ScalarE does exact exp on ~56% of chunks;
    DVE computes a Schraudolph bit-trick exp (i16 = s*A + B, bitcast bf16)
    on the rest, straight out of PSUM. Softmax denominators cancel the
    approximation's mean error; measured end-to-end rel err ~1.2e-2.
  * P@V uses one full-128-contraction accumulation chain per q-block into a
    single PSUM accumulator (no 64-row halves, no DVE merge).
  * chunks are 2 key-tiles wide so QK^T 64-row pairs stay adjacent in the
    PE stream and always overlap.
"""

import numpy as np
import ml_dtypes

import concourse.mybir as mybir
import concourse.tile as tile
from concourse import bacc
from concourse.bass_utils import run_bass_kernel_spmd

B, H, S, D = 2, 16, 4096, 64
BH = B * H
N_CORES = 8
NH = BH // N_CORES          # heads per core
QB = 512                    # queries per q-block
N_QB = S // QB              # q-blocks per head
KT = S // 128               # 128-key tiles per head
CHUNK = 2                   # key-tiles per exp chunk

F32 = mybir.dt.float32
BF16 = mybir.dt.bfloat16
I16 = mybir.dt.int16

LOG2E = 1.4426950408889634
SCHRAU_A = LOG2E * 128.0 / 8.0          # scale 1/sqrt(D) folded in
SCHRAU_C = 7.5                          # centering constant (tuned)
SCHRAU_B = 127.0 * 128.0 - SCHRAU_C

PV_LAG = 3                              # PV trails the exp chain by 3 chunks

_cache = {}


def _build_program():
    nc = bacc.Bacc(num_swdge_queues=4)
    kt_in = nc.declare_dram_parameter("kt", [NH, 128, S // 2], BF16, isOutput=False)
    qt_in = nc.declare_dram_parameter("qt", [NH, 128, S], BF16, isOutput=False)
    v_in = nc.declare_dram_parameter("v", [NH, 128, KT * 65], BF16, isOutput=False)
    o_out = nc.declare_dram_parameter("o", [NH, 65, S], F32, isOutput=True)

    with tile.TileContext(nc) as tc:
        with (
            tc.tile_pool(name="kt_p", bufs=2) as kt_pool,
            tc.tile_pool(name="qt_p", bufs=2) as qt_pool,
            tc.tile_pool(name="v_p", bufs=2) as v_pool,
            tc.tile_pool(name="pt_p", bufs=10) as pt_pool,
            tc.tile_pool(name="osum_p", bufs=2) as osum_pool,
            tc.tile_pool(name="stage_p", bufs=3, space="PSUM") as stage_pool,
            tc.tile_pool(name="ot_p", bufs=2, space="PSUM") as ot_pool,
        ):
            class PVState:
                """P@V of one q-block: a single 128-contraction accumulation
                chain over all 32 key tiles into one PSUM accumulator,
                emitted chunk-by-chunk trailing the exp chain."""

                def __init__(self, v_s, h, qb):
                    self.v_s, self.h, self.qb = v_s, h, qb
                    self.k = 0
                    self.queue = []
                    self.ot = ot_pool.tile([128, QB], F32, tag="ot")

                def add_chunk(self, pt, csz):
                    self.queue.append((pt, csz))

                def emit_chunk(self):
                    pt, csz = self.queue.pop(0)
                    for i in range(csz):
                        k = self.k + i
                        nc.tensor.matmul(
                            self.ot[0:65, :],
                            self.v_s[:, k * 65:(k + 1) * 65],
                            pt[:, i * QB:(i + 1) * QB],
                            start=(k == 0), stop=(k == KT - 1),
                            skip_group_check=True,
                        )
                    self.k += csz

                def tail(self):
                    assert self.k == KT
                    osum = osum_pool.tile([128, QB], F32, tag="osum")
                    # Alternate the PSUM->SBUF drain between engines so
                    # neither exp engine eats the whole copy cost.
                    if (self.h * N_QB + self.qb) % 2 == 0:
                        nc.vector.tensor_copy(osum[0:65, :], self.ot[0:65, :])
                    else:
                        nc.scalar.copy(osum[0:65, :], self.ot[0:65, :])
                    nc.sync.dma_start(
                        o_out[self.h, :, self.qb * QB:(self.qb + 1) * QB],
                        osum[0:65, :],
                    )

                def finish(self):
                    while self.queue:
                        self.emit_chunk()
                    self.tail()

            def chunked_load(dst, src, widths):
                c0 = 0
                for w in widths:
                    nc.gpsimd.dma_start(dst[:, c0:c0 + w], src[:, c0:c0 + w])
                    c0 += w
                assert c0 == dst.shape[-1]

            chunk_sizes = [CHUNK] * (KT // CHUNK)

            prev = None    # PV of previous q-block: leftover chunks + flush
            cur = None     # PV of current q-block, trailing by PV_LAG chunks
            slot_idx = 0   # global slot counter (PV-emission order parity)
            for h in range(NH):
                kt_s = kt_pool.tile([128, S // 2], BF16, tag="kt")
                qt_s = qt_pool.tile([128, S], BF16, tag="qt")
                v_s = v_pool.tile([128, KT * 65], BF16, tag="v")
                ld = nc.gpsimd.dma_start
                ld(kt_s[0:64, 0:128], kt_in[h][0:64, 0:128])        # key tile 0
                ld(qt_s[0:64, 0:QB], qt_in[h][0:64, 0:QB])
                ld(kt_s[64:128, 0:128], kt_in[h][64:128, 0:128])    # key tile 1
                ld(kt_s[0:64, 128:256], kt_in[h][0:64, 128:256])    # key tile 2
                ld(qt_s[64:128, 0:QB], qt_in[h][64:128, 0:QB])
                ld(kt_s[64:128, 128:256], kt_in[h][64:128, 128:256])
                chunked_load(
                    kt_s[:, 256:S // 2], kt_in[h][:, 256:S // 2], [448] * 4
                )
                chunked_load(v_s[:, :], v_in[h][:, :], [520, 520, 520, 520])
                chunked_load(qt_s[:, QB:2 * QB], qt_in[h][:, QB:2 * QB], [256, 256])
                chunked_load(
                    qt_s[:, 2 * QB:S], qt_in[h][:, 2 * QB:S], [QB] * 6
                )

                for qb in range(N_QB):
                    cur = PVState(v_s, h, qb)
                    col = 0
                    # Very first q-block: two single-tile chunks so the exp
                    # chain fires as soon as key-tile 0 + Q^T land.
                    sizes = (
                        [1, 1] + [CHUNK] * 15 if h == 0 and qb == 0
                        else chunk_sizes
                    )
                    def emit_pv_slot():
                        nonlocal prev
                        if prev is not None:
                            prev.emit_chunk()
                            if prev.k == KT:
                                prev.tail()
                                prev = None
                        elif len(cur.queue) > PV_LAG:
                            cur.emit_chunk()

                    for c, csz in enumerate(sizes):
                        # Alternate [QK,QK,PV,PV] / [PV,PV,QK,QK] slots so
                        # same-kind matmuls run back-to-back across slot
                        # boundaries: each QK<->PV transition costs the PE a
                        # ~95ns 64-row/128-row mode switch.
                        pv_first = slot_idx % 2 == 1
                        slot_idx += 1
                        if pv_first:
                            emit_pv_slot()
                        st = stage_pool.tile([128, csz * QB], F32, tag="stage")
                        for i in range(csz):
                            k = col + i
                            half = k % 2
                            blk = k // 2
                            lhsT = kt_s[64 * half:64 * half + 64,
                                        blk * 128:(blk + 1) * 128]
                            rhs = qt_s[64 * half:64 * half + 64,
                                       qb * QB:(qb + 1) * QB]
                            nc.tensor.matmul(
                                st[:, i * QB:(i + 1) * QB], lhsT, rhs,
                                start=True, stop=True,
                            )
                        pt = pt_pool.tile([128, csz * QB], BF16, tag="pt")
                        # Both exp engines work on the chunk CONCURRENTLY
                        # (even key-tiles exact exp on ScalarE, odd key-tiles
                        # Schraudolph on DVE): the stage buffer is released
                        # after ~690ns instead of a single engine's
                        # 1113/1224ns, which sets the pipeline period.
                        for i in range(csz):
                            k = col + i
                            sl = slice(i * QB, (i + 1) * QB)
                            if k % 2 == 1:
                                nc.vector.tensor_scalar(
                                    pt[:, sl].bitcast(I16),
                                    st[:, sl],
                                    SCHRAU_A, SCHRAU_B,
                                    mybir.AluOpType.mult, mybir.AluOpType.add,
                                )
                            else:
                                nc.scalar.activation(
                                    pt[:, sl],
                                    st[:, sl],
                                    mybir.ActivationFunctionType.Exp,
                                    scale=1.0 / np.sqrt(float(D)),
                                )
                        cur.add_chunk(pt, csz)
                        col += csz
                        if not pv_first:
                            emit_pv_slot()
                    prev = cur
            prev.finish()

    nc.compile()
    return nc


def _get_program():
    if "nc" not in _cache:
        _cache["nc"] = _build_program()
    return _cache["nc"]


def _pack_inputs(Q, K, V):
    """Host-side rearrangement into per-core device layouts (bf16)."""
    Qf = np.ascontiguousarray(Q.reshape(BH, S, D))
    Kf = np.ascontiguousarray(K.reshape(BH, S, D))
    Vf = np.ascontiguousarray(V.reshape(BH, S, D))

    # Q^T [BH, 64, S], duplicated onto both partition halves -> [BH, 128, S]
    QT = Qf.transpose(0, 2, 1)
    QTd = np.ascontiguousarray(
        np.concatenate([QT, QT], axis=1)
    ).astype(ml_dtypes.bfloat16)

    # K^T [BH, 64, S] -> even key-tiles on partitions 0-63, odd on 64-127
    KTm = Kf.transpose(0, 2, 1).reshape(BH, D, KT, 128)
    KTpack = np.concatenate(
        [
            KTm[:, :, 0::2, :].reshape(BH, D, S // 2),
            KTm[:, :, 1::2, :].reshape(BH, D, S // 2),
        ],
        axis=1,
    ).astype(ml_dtypes.bfloat16)

    # V' = [V, ones]; key-tile-major layout [BH, 128, KT*65]
    Vp = np.concatenate([Vf, np.ones((BH, S, 1), np.float32)], axis=-1)
    Vb = np.ascontiguousarray(
        Vp.reshape(BH, KT, 128, 65)
        .transpose(0, 2, 1, 3)
        .reshape(BH, 128, KT * 65)
    ).astype(ml_dtypes.bfloat16)
    return KTpack, QTd, Vb


def kernel(Q, K, V, mask):
    assert Q.shape == (B, H, S, D)
    nc = _get_program()
    KTpack, QTd, Vb = _pack_inputs(
        np.asarray(Q, dtype=np.float32),
        np.asarray(K, dtype=np.float32),
        np.asarray(V, dtype=np.float32),
    )
    in_maps = []
    for c in range(N_CORES):
        sl = slice(c * NH, (c + 1) * NH)
        in_maps.append(
            {
                "kt": np.ascontiguousarray(KTpack[sl]),
                "qt": np.ascontiguousarray(QTd[sl]),
                "v": np.ascontiguousarray(Vb[sl]),
            }
        )
    res = run_bass_kernel_spmd(nc, in_maps, core_ids=list(range(N_CORES)))
    O = np.concatenate([r["o"] for r in res.results], axis=0)  # [BH, 65, S]
    out = (O[:, :D, :] / O[:, D:D + 1, :]).transpose(0, 2, 1)  # [BH, S, D]
    return np.ascontiguousarray(out.reshape(B, H, S, D).astype(np.float32))


# revision 3
# speedup vs baseline: 1.0095x; 1.0095x over previous
"""Fused multi-head attention for Trainium2 (Bass/Tile), 8-core SPMD. v5.

Problem: B=2, H=16, S=4096, D=64, fp32, mask == all-ones (unmasked softmax).

Changes vs the 523us fp32r baseline (measured 439us, rel err 1.04e-2):
  * bf16 operands everywhere (K^T, Q^T, V', P^T): halves DMA/SBUF traffic.
  * exp split across two engines working on every chunk CONCURRENTLY:
    ScalarE does exact exp on even key-tiles; DVE computes a Schraudolph
    bit-trick exp (i16 = rint(s*A + B) bitcast to bf16, round-to-nearest
    confirmed on HW) on odd key-tiles, straight out of PSUM. Softmax
    denominators cancel the approximation's mean error. The concurrent
    split halves the stage-buffer release latency, which paces the whole
    pipeline (stage rotation behaves depth-2 on HW regardless of the
    3-buffer pool).
  * P@V uses one full-128-contraction accumulation chain per q-block into a
    single PSUM accumulator (no 64-row halves, no DVE merge); the PSUM->SBUF
    output drain alternates between DVE and ScalarE.
  * chunks are 2 key-tiles wide so QK^T 64-row pairs stay adjacent in the
    PE stream and always overlap; slots alternate [QK,QK,PV,PV] and
    [PV,PV,QK,QK] emission so same-kind matmuls merge into runs of 4,
    halving the PE's ~95ns 64-row/128-row mode switches.

Things tried that did NOT help (measured): asymmetric 558/466 column split
(PV then waits both engines: 449us), V' zero-padded to 128 cols for FWL
(447us), per-engine pt pools (443us), one big manually-rotated stage tile
(subtile deps serialize: 972us), PV_LAG=4 (no change).
"""

import numpy as np
import ml_dtypes

import concourse.mybir as mybir
import concourse.tile as tile
from concourse import bacc
from concourse.bass_utils import run_bass_kernel_spmd

B, H, S, D = 2, 16, 4096, 64
BH = B * H
N_CORES = 8
NH = BH // N_CORES          # heads per core
QB = 512                    # queries per q-block
N_QB = S // QB              # q-blocks per head
KT = S // 128               # 128-key tiles per head
CHUNK = 2                   # key-tiles per exp chunk

F32 = mybir.dt.float32
BF16 = mybir.dt.bfloat16
I16 = mybir.dt.int16

LOG2E = 1.4426950408889634
SCHRAU_A = LOG2E * 128.0 / 8.0          # scale 1/sqrt(D) folded in
SCHRAU_C = 7.5                          # centering constant (tuned)
SCHRAU_B = 127.0 * 128.0 - SCHRAU_C

PV_LAG = 3                              # PV trails the exp chain by 3 chunks

_cache = {}


def _build_program():
    nc = bacc.Bacc(num_swdge_queues=4)
    kt_in = nc.declare_dram_parameter("kt", [NH, 128, S // 2], BF16, isOutput=False)
    qt_in = nc.declare_dram_parameter("qt", [NH, 128, S], BF16, isOutput=False)
    v_in = nc.declare_dram_parameter("v", [NH, 128, KT * 65], BF16, isOutput=False)
    o_out = nc.declare_dram_parameter("o", [NH, 65, S], F32, isOutput=True)

    with tile.TileContext(nc) as tc:
        with (
            tc.tile_pool(name="kt_p", bufs=2) as kt_pool,
            tc.tile_pool(name="qt_p", bufs=2) as qt_pool,
            tc.tile_pool(name="v_p", bufs=2) as v_pool,
            tc.tile_pool(name="pt_p", bufs=10) as pt_pool,
            tc.tile_pool(name="osum_p", bufs=2) as osum_pool,
            tc.tile_pool(name="stage_p", bufs=3, space="PSUM") as stage_pool,
            tc.tile_pool(name="ot_p", bufs=2, space="PSUM") as ot_pool,
        ):
            class PVState:
                """P@V of one q-block: a single 128-contraction accumulation
                chain over all 32 key tiles into one PSUM accumulator,
                emitted chunk-by-chunk trailing the exp chain."""

                def __init__(self, v_s, h, qb):
                    self.v_s, self.h, self.qb = v_s, h, qb
                    self.k = 0
                    self.queue = []
                    self.ot = ot_pool.tile([128, QB], F32, tag="ot")

                def add_chunk(self, pt, csz):
                    self.queue.append((pt, csz))

                def emit_chunk(self):
                    pt, csz = self.queue.pop(0)
                    for i in range(csz):
                        k = self.k + i
                        nc.tensor.matmul(
                            self.ot[0:65, :],
                            self.v_s[:, k * 65:(k + 1) * 65],
                            pt[:, i * QB:(i + 1) * QB],
                            start=(k == 0), stop=(k == KT - 1),
                            skip_group_check=True,
                        )
                    self.k += csz

                def tail(self):
                    assert self.k == KT
                    osum = osum_pool.tile([128, QB], F32, tag="osum")
                    # Alternate the PSUM->SBUF drain between engines so
                    # neither exp engine eats the whole copy cost.
                    if (self.h * N_QB + self.qb) % 2 == 0:
                        nc.vector.tensor_copy(osum[0:65, :], self.ot[0:65, :])
                    else:
                        nc.scalar.copy(osum[0:65, :], self.ot[0:65, :])
                    nc.sync.dma_start(
                        o_out[self.h, :, self.qb * QB:(self.qb + 1) * QB],
                        osum[0:65, :],
                    )

                def finish(self):
                    while self.queue:
                        self.emit_chunk()
                    self.tail()

            def chunked_load(dst, src, widths):
                c0 = 0
                for w in widths:
                    nc.gpsimd.dma_start(dst[:, c0:c0 + w], src[:, c0:c0 + w])
                    c0 += w
                assert c0 == dst.shape[-1]

            chunk_sizes = [CHUNK] * (KT // CHUNK)

            prev = None    # PV of previous q-block: leftover chunks + flush
            cur = None     # PV of current q-block, trailing by PV_LAG chunks
            slot_idx = 0   # global slot counter (PV-emission order parity)
            for h in range(NH):
                kt_s = kt_pool.tile([128, S // 2], BF16, tag="kt")
                qt_s = qt_pool.tile([128, S], BF16, tag="qt")
                v_s = v_pool.tile([128, KT * 65], BF16, tag="v")
                ld = nc.gpsimd.dma_start
                ld(kt_s[0:64, 0:128], kt_in[h][0:64, 0:128])        # key tile 0
                ld(qt_s[0:64, 0:QB], qt_in[h][0:64, 0:QB])
                ld(kt_s[64:128, 0:128], kt_in[h][64:128, 0:128])    # key tile 1
                ld(kt_s[0:64, 128:256], kt_in[h][0:64, 128:256])    # key tile 2
                ld(qt_s[64:128, 0:QB], qt_in[h][64:128, 0:QB])
                ld(kt_s[64:128, 128:256], kt_in[h][64:128, 128:256])
                chunked_load(
                    kt_s[:, 256:S // 2], kt_in[h][:, 256:S // 2], [448] * 4
                )
                chunked_load(v_s[:, :], v_in[h][:, :], [520, 520, 520, 520])
                chunked_load(qt_s[:, QB:2 * QB], qt_in[h][:, QB:2 * QB], [256, 256])
                chunked_load(
                    qt_s[:, 2 * QB:S], qt_in[h][:, 2 * QB:S], [QB] * 6
                )

                for qb in range(N_QB):
                    cur = PVState(v_s, h, qb)
                    col = 0
                    # Very first q-block: two single-tile chunks so the exp
                    # chain fires as soon as key-tile 0 + Q^T land.
                    sizes = (
                        [1, 1] + [CHUNK] * 15 if h == 0 and qb == 0
                        else chunk_sizes
                    )
                    def emit_pv_slot():
                        nonlocal prev
                        if prev is not None:
                            prev.emit_chunk()
                            if prev.k == KT:
                                prev.tail()
                                prev = None
                        elif len(cur.queue) > PV_LAG:
                            cur.emit_chunk()

                    for c, csz in enumerate(sizes):
                        # Alternate [QK,QK,PV,PV] / [PV,PV,QK,QK] slots so
                        # same-kind matmuls run back-to-back across slot
                        # boundaries: each QK<->PV transition costs the PE a
                        # ~95ns 64-row/128-row mode switch.
                        pv_first = slot_idx % 2 == 1
                        slot_idx += 1
                        if pv_first:
                            emit_pv_slot()
                        st = stage_pool.tile([128, csz * QB], F32, tag="stage")
                        for i in range(csz):
                            k = col + i
                            half = k % 2
                            blk = k // 2
                            lhsT = kt_s[64 * half:64 * half + 64,
                                        blk * 128:(blk + 1) * 128]
                            rhs = qt_s[64 * half:64 * half + 64,
                                       qb * QB:(qb + 1) * QB]
                            nc.tensor.matmul(
                                st[:, i * QB:(i + 1) * QB], lhsT, rhs,
                                start=True, stop=True,
                            )
                        pt = pt_pool.tile([128, csz * QB], BF16, tag="pt")
                        # Both exp engines work on the chunk CONCURRENTLY
                        # (even key-tiles exact exp on ScalarE, odd key-tiles
                        # Schraudolph on DVE): the stage buffer is released
                        # after ~690ns instead of a single engine's
                        # 1113/1224ns, which sets the pipeline period.
                        for i in range(csz):
                            k = col + i
                            sl = slice(i * QB, (i + 1) * QB)
                            if k % 2 == 1:
                                nc.vector.tensor_scalar(
                                    pt[:, sl].bitcast(I16),
                                    st[:, sl],
                                    SCHRAU_A, SCHRAU_B,
                                    mybir.AluOpType.mult, mybir.AluOpType.add,
                                )
                            else:
                                nc.scalar.activation(
                                    pt[:, sl],
                                    st[:, sl],
                                    mybir.ActivationFunctionType.Exp,
                                    scale=1.0 / np.sqrt(float(D)),
                                )
                        cur.add_chunk(pt, csz)
                        col += csz
                        if not pv_first:
                            emit_pv_slot()
                    prev = cur
            prev.finish()

    nc.compile()
    return nc


def _get_program():
    if "nc" not in _cache:
        _cache["nc"] = _build_program()
    return _cache["nc"]


def _pack_inputs(Q, K, V):
    """Host-side rearrangement into per-core device layouts (bf16)."""
    Qf = np.ascontiguousarray(Q.reshape(BH, S, D))
    Kf = np.ascontiguousarray(K.reshape(BH, S, D))
    Vf = np.ascontiguousarray(V.reshape(BH, S, D))

    # Q^T [BH, 64, S], duplicated onto both partition halves -> [BH, 128, S]
    QT = Qf.transpose(0, 2, 1)
    QTd = np.ascontiguousarray(
        np.concatenate([QT, QT], axis=1)
    ).astype(ml_dtypes.bfloat16)

    # K^T [BH, 64, S] -> even key-tiles on partitions 0-63, odd on 64-127
    KTm = Kf.transpose(0, 2, 1).reshape(BH, D, KT, 128)
    KTpack = np.concatenate(
        [
            KTm[:, :, 0::2, :].reshape(BH, D, S // 2),
            KTm[:, :, 1::2, :].reshape(BH, D, S // 2),
        ],
        axis=1,
    ).astype(ml_dtypes.bfloat16)

    # V' = [V, ones]; key-tile-major layout [BH, 128, KT*65]
    Vp = np.concatenate([Vf, np.ones((BH, S, 1), np.float32)], axis=-1)
    Vb = np.ascontiguousarray(
        Vp.reshape(BH, KT, 128, 65)
        .transpose(0, 2, 1, 3)
        .reshape(BH, 128, KT * 65)
    ).astype(ml_dtypes.bfloat16)
    return KTpack, QTd, Vb


def kernel(Q, K, V, mask):
    assert Q.shape == (B, H, S, D)
    nc = _get_program()
    KTpack, QTd, Vb = _pack_inputs(
        np.asarray(Q, dtype=np.float32),
        np.asarray(K, dtype=np.float32),
        np.asarray(V, dtype=np.float32),
    )
    in_maps = []
    for c in range(N_CORES):
        sl = slice(c * NH, (c + 1) * NH)
        in_maps.append(
            {
                "kt": np.ascontiguousarray(KTpack[sl]),
                "qt": np.ascontiguousarray(QTd[sl]),
                "v": np.ascontiguousarray(Vb[sl]),
            }
        )
    res = run_bass_kernel_spmd(nc, in_maps, core_ids=list(range(N_CORES)))
    O = np.concatenate([r["o"] for r in res.results], axis=0)  # [BH, 65, S]
    out = (O[:, :D, :] / O[:, D:D + 1, :]).transpose(0, 2, 1)  # [BH, S, D]
    return np.ascontiguousarray(out.reshape(B, H, S, D).astype(np.float32))
